# revision 1
# baseline (speedup 1.0000x reference)
"""DeepseekV2 MoE (T=2048, H=2048, E=16 experts, top-6, I=1408, shared IS=2816)
on 8 Trainium2 NeuronCores.

Strategy (expert-parallel per the sharding hint):
  - Host: gate softmax/top-6 (numpy replica of the reference; the top-6/7
    logit gap is ~7e-5 so the selection is rounding-robust), per-expert token
    gather, weight transpose/packing into DMA-friendly layouts, and the final
    scatter/combine (0.05% of the FLOPs).
  - Device (SPMD, 8 cores): core i owns routed experts 2i and 2i+1 (dense
    SwiGLU over a fixed capacity C=896 of gathered tokens, combine weights
    applied on-chip) plus 1/8 of the shared expert (tensor-parallel over the
    intermediate dim, 352 padded to 384). All matmuls run in float32r
    (FP22 truncation — full PE throughput at free-dim >= 256).
  - No collectives: per-core outputs are disjoint (routed) or partial sums
    (shared) that the host adds.
"""

import os
import numpy as np

import concourse.bass as bass
import concourse.mybir as mybir
import concourse.tile as tile
from concourse.bass_utils import run_bass_kernel_spmd

F32 = mybir.dt.float32
F32R = mybir.dt.float32r
AF = mybir.ActivationFunctionType

# problem dims (hardcoded per spec)
T, H, I, E, TOP_K = 2048, 2048, 1408, 16, 6
FF = 2 * I              # 2816
IS = 2 * I              # shared intermediate
N_CORES = 8
C = 896                 # per-expert token capacity (binomial mean 768, sd ~22;
                        # seed-0 max count is 818; overflow falls back to host)
ISP = 384               # per-core shared-intermediate slice, 352 padded to 384

HC = H // 128           # 16 H chunks (contraction for mm1)
IC = I // 128           # 11 I chunks (contraction for mm2)
HB = H // 512           # 4 output H blocks
KS = ISP // 128         # 3 shared-intermediate chunks
TBP = T // 1024         # 2 shared token super-blocks (1024 tokens each)


def _mm_blocks(width):
    """Moving-dim blocks of 512 with a >=256 tail (fp32r full rate needs >=256)."""
    out, off = [], 0
    while off < width:
        w = min(512, width - off)
        assert w >= 256
        out.append((off, w))
        off += w
    return out


def _split_excess_waits(nc, cap=1):
    """This container's walrus accepts at most one semaphore wait per
    instruction; move excess waits onto inserted same-engine NOPs."""
    for bb in nc.main_func.blocks:
        new_list = []
        for ins in bb.instructions:
            si = getattr(ins, "sync_info", None)
            waits = list(si.on_wait) if (si is not None and si.on_wait) else []
            if len(waits) > cap:
                excess, keep = waits[:-cap], waits[-cap:]
                si.on_wait = keep
                for i in range(0, len(excess), cap):
                    nop = mybir.InstNoOp(
                        name=f"I-waitsplit-{nc.next_id()}",
                        engine=ins.engine,
                        ins=[],
                        outs=[],
                        sync_info=mybir.SyncInfo(
                            on_update=[], on_wait=excess[i : i + cap]
                        ),
                        bass_nofuse=True,
                    )
                    nc.register_instruction(nop, overwrite=True)
                    new_list.append(nop)
            new_list.append(ins)
        bb.instructions = new_list


def build_nc(cap: int):
    """Build the per-core Bass program for token capacity `cap` (mult of 512)."""
    cc_n = cap // 128
    nb_n = cap // 512
    nc = bass.Bass()

    # --- DRAM parameters (packed layouts; partition dim = 128 first) ---
    # x.T gathered per owned expert: [slot][128p(H in), HC, cap]
    xt_d = [
        nc.declare_dram_parameter(f"xt{s}", [128, HC, cap], F32R, isOutput=False)
        for s in range(2)
    ]
    # w13[e].T blocks, order g0,u0,g1,u1,...: [2*IC][128p(H in), HC, 128]
    w13_d = [
        nc.declare_dram_parameter(f"w13_{s}", [2 * IC, 128, HC, 128], F32R, isOutput=False)
        for s in range(2)
    ]
    # w2[e].T blocks per output H block: [HB][128p(I in), IC, 512]
    w2_d = [
        nc.declare_dram_parameter(f"w2_{s}", [HB, 128, IC, 512], F32R, isOutput=False)
        for s in range(2)
    ]
    # x.T full (shared expert), token super-blocks: [TBP][128p(H in), HC, 1024]
    xts_d = nc.declare_dram_parameter("xts", [TBP, 128, HC, 1024], F32R, isOutput=False)
    # shared w13 slice blocks (g0,u0,g1,u1,g2,u2): [6][128p(H in), HC, 128]
    sw13_d = nc.declare_dram_parameter("sw13", [2 * KS, 128, HC, 128], F32R, isOutput=False)
    # shared w2 slice blocks: [HB][128p(ISP in), KS, 512]
    sw2_d = nc.declare_dram_parameter("sw2", [HB, 128, KS, 512], F32R, isOutput=False)
    # combine weights: [128, 2 * cc_n] (col s*cc_n+cc -> tokens cc*128..+128 of slot s)
    c_d = nc.declare_dram_parameter("cvec", [128, 2 * cc_n], F32, isOutput=False)

    yout_d = [
        nc.declare_dram_parameter(f"yout{s}", [cap, H], F32, isOutput=True)
        for s in range(2)
    ]
    ys_d = nc.declare_dram_parameter("ys", [T, H], F32, isOutput=True)

    with tile.TileContext(nc) as tc:
        with (
            tc.tile_pool(name="xt", bufs=1) as p_xt,
            tc.tile_pool(name="w13", bufs=3) as p_w13,
            tc.tile_pool(name="w2", bufs=2) as p_w2,
            tc.tile_pool(name="tmp", bufs=3) as p_tmp,
            tc.tile_pool(name="aT", bufs=1) as p_aT,
            tc.tile_pool(name="y", bufs=4) as p_y,
            tc.tile_pool(name="c", bufs=1) as p_c,
            tc.tile_pool(name="ps", bufs=8, space="PSUM") as p_ps,
        ):
            c_sb = p_c.tile([128, 2 * cc_n], F32)
            nc.sync.dma_start(out=c_sb[:], in_=c_d[:])

            def load_xt(dram_src, width):
                """Load an x.T block with per-H-chunk strip DMAs, ordered so
                the first 512-wide block (what the first PSUM accumulation
                group consumes) lands first."""
                t = p_xt.tile([128, HC, width], F32R, tag="xt")
                for off, w in _mm_blocks(width):
                    for hc in range(HC):
                        nc.sync.dma_start(
                            out=t[:, hc, off:off + w],
                            in_=dram_src[:, hc, off:off + w],
                        )
                return t

            def swiglu_mm1(xt_sb, w13_src, n_i, aT_sb, width):
                """mm1 + SiLU*u for one weight set.
                xt_sb: [128, HC, width]; w13_src: DRAM [2*n_i, 128, HC, 128];
                aT_sb: [128, n_i, width] destination (f32r)."""
                for i in range(n_i):
                    wg = p_w13.tile([128, HC, 128], F32R, tag="w13")
                    nc.sync.dma_start(out=wg[:], in_=w13_src[2 * i])
                    wu = p_w13.tile([128, HC, 128], F32R, tag="w13")
                    nc.sync.dma_start(out=wu[:], in_=w13_src[2 * i + 1])
                    for off, w in _mm_blocks(width):
                        col = slice(off, off + w)
                        ps_g = p_ps.tile([128, 512], F32, tag="ps")
                        ps_u = p_ps.tile([128, 512], F32, tag="ps")
                        for hc in range(HC):
                            nc.tensor.matmul(
                                ps_g[:, :w], wg[:, hc, :], xt_sb[:, hc, col],
                                start=(hc == 0), stop=(hc == HC - 1),
                            )
                        for hc in range(HC):
                            nc.tensor.matmul(
                                ps_u[:, :w], wu[:, hc, :], xt_sb[:, hc, col],
                                start=(hc == 0), stop=(hc == HC - 1),
                            )
                        tmp = p_tmp.tile([128, 512], F32, tag="tmp")
                        nc.scalar.activation(
                            out=tmp[:, :w], in_=ps_g[:, :w], func=AF.Silu
                        )
                        nc.vector.tensor_mul(
                            out=aT_sb[:, i, col], in0=tmp[:, :w], in1=ps_u[:, :w]
                        )

            # One shared-expert token super-block (1024 tokens, 1/8 TP slice)
            def shared_phase(tbp):
                xts_sb = load_xt(xts_d[tbp], 1024)

                aTs = p_aT.tile([128, KS, 1024], F32R, tag="aT")
                swiglu_mm1(xts_sb, sw13_d, KS, aTs, 1024)

                for hb in range(HB):
                    sw2b = p_w2.tile([128, KS, 512], F32R, tag="w2")
                    nc.sync.dma_start(out=sw2b[:], in_=sw2_d[hb])
                    for tc_ in range(8):
                        ps_y = p_ps.tile([128, 512], F32, tag="ps")
                        for k in range(KS):
                            nc.tensor.matmul(
                                ps_y[:],
                                aTs[:, k, tc_ * 128:(tc_ + 1) * 128],
                                sw2b[:, k, :],
                                start=(k == 0), stop=(k == KS - 1),
                            )
                        y_sb = p_y.tile([128, 512], F32, tag="y")
                        nc.vector.tensor_copy(y_sb[:], ps_y[:])
                        row0 = tbp * 1024 + tc_ * 128
                        nc.gpsimd.dma_start(
                            out=ys_d[row0:row0 + 128, hb * 512:(hb + 1) * 512],
                            in_=y_sb[:],
                        )

            # One routed expert (dense over the capacity token set)
            def expert_phase(s):
                xt_sb = load_xt(xt_d[s], cap)

                aT = p_aT.tile([128, IC, cap], F32R, tag="aT")
                swiglu_mm1(xt_sb, w13_d[s], IC, aT, cap)

                # mm2: y[c,h] = sum_i a[c,i] * w2T[i,h], c-scaled on evac
                for hb in range(HB):
                    w2b = p_w2.tile([128, IC, 512], F32R, tag="w2")
                    nc.sync.dma_start(out=w2b[:], in_=w2_d[s][hb])
                    for cc in range(cc_n):
                        ps_y = p_ps.tile([128, 512], F32, tag="ps")
                        for ic in range(IC):
                            nc.tensor.matmul(
                                ps_y[:],
                                aT[:, ic, cc * 128:(cc + 1) * 128],
                                w2b[:, ic, :],
                                start=(ic == 0), stop=(ic == IC - 1),
                            )
                        y_sb = p_y.tile([128, 512], F32, tag="y")
                        nc.vector.tensor_scalar_mul(
                            y_sb[:], ps_y[:], c_sb[:, s * cc_n + cc : s * cc_n + cc + 1]
                        )
                        nc.gpsimd.dma_start(
                            out=yout_d[s][cc * 128:(cc + 1) * 128,
                                          hb * 512:(hb + 1) * 512],
                            in_=y_sb[:],
                        )

            # Interleave: each phase's loads prefetch under the previous
            # phase's compute instead of colliding at phase boundaries.
            shared_phase(0)
            expert_phase(0)
            shared_phase(1)
            expert_phase(1)

    _split_excess_waits(nc, cap=1)
    return nc


# ------------------------- host side -------------------------

def _gate_combine(x, gate_w):
    """Replica of the reference gate in pure numpy (f32). The top-6 selection
    is what must match the reference exactly; the smallest rank-6/rank-7 logit
    gap over the 2048 tokens is ~7e-5 while cross-implementation f32 rounding
    differences are ~1e-6, so the selection is identical. Tie-break on exact
    equality follows lax.top_k (lowest index wins)."""
    z = (x @ gate_w.T).astype(np.float32)                 # [T, E] logits
    z64 = z.astype(np.float64)
    m = z64.max(-1, keepdims=True)
    ez = np.exp(z64 - m)
    scores = (ez / ez.sum(-1, keepdims=True)).astype(np.float32)
    # top-6 per token, ties broken by lowest expert index (argsort on
    # (-score, idx) via stable mergesort on -score)
    order = np.argsort(-scores, axis=-1, kind="stable")[:, :TOP_K]
    topk_w = np.take_along_axis(scores, order, axis=-1)
    topk_w = topk_w / (topk_w.sum(-1, keepdims=True) + 1e-20)
    combine = np.zeros((x.shape[0], E), np.float32)
    np.put_along_axis(combine, order, topk_w, axis=-1)
    return combine


def _pack_w13(w13e):
    """w13[e] [FF, H] -> [2*IC, 128, HC, 128] with block order g0,u0,g1,u1,..."""
    # w13e.T is [H, FF]; block j covers FF cols j*128..(j+1)*128
    # reshape w13e [FF, H] = [2*IC jb, 128 f, HC o, 128 p] -> [jb, p, o, f]
    a = np.ascontiguousarray(
        w13e.reshape(2 * IC, 128, HC, 128).transpose(0, 3, 2, 1)
    )
    order = np.empty(2 * IC, np.int64)
    order[0::2] = np.arange(IC)           # gate chunks 0..10
    order[1::2] = np.arange(IC) + IC      # up chunks 11..21
    return np.ascontiguousarray(a[order])


def _pack_w2(w2e):
    """w2[e] [H, I] -> [HB, 128, IC, 512]: w2T[i, h] with i=o*128+p, h=hb*512+f."""
    return np.ascontiguousarray(
        w2e.reshape(HB, 512, IC, 128).transpose(0, 3, 2, 1)
    )


def _pack_xT(xT, width):
    """xT [H, n*width] -> [n, 128, HC, width]"""
    n = xT.shape[1] // width
    return np.ascontiguousarray(
        xT.reshape(HC, 128, n, width).transpose(2, 1, 0, 3)
    )


def _host_moe(x, combine, w13, w2, sw13, sw2):
    """Exact numpy fallback (only used on absurd routing imbalance)."""

    def silu(v):
        return v / (1.0 + np.exp(-v))

    out = np.zeros((T, H), np.float32)
    for e in range(E):
        gu = x @ w13[e].T
        a = silu(gu[:, :I]) * gu[:, I:]
        out += combine[:, e:e + 1] * (a @ w2[e].T)
    gu = x @ sw13.T
    a = silu(gu[:, :IS]) * gu[:, IS:]
    out += a @ sw2.T
    return out


_NC_CACHE = {}

LAST_EXEC_TIME_NS = None
LAST_TRACE = None


def _install_ntff_hook():
    """Bridge the missing ``antenv.axon_hooks`` module so trace=True works
    in this container (used by test.py only; harmless if already present)."""
    import sys, types

    try:
        from antenv.axon_hooks import get_axon_ntff_profile_hook  # noqa: F401
        return
    except ImportError:
        pass
    import antenv  # noqa: F401
    import trn_agent_boot.trn_boot as tb

    mod = types.ModuleType("antenv.axon_hooks")
    _h = [None]
    mod.set_axon_ntff_profile_hook = lambda h: _h.__setitem__(0, h)
    mod.get_axon_ntff_profile_hook = lambda: _h[0]
    sys.modules["antenv.axon_hooks"] = mod
    mod.set_axon_ntff_profile_hook(
        tb._ntff_profile_via_ctypes("/opt/axon/libaxon_pjrt.so")
    )


def kernel(hidden_states, gate_w, w13, w2, sw13, sw2):
    hidden_states = np.asarray(hidden_states)
    x = np.ascontiguousarray(hidden_states.reshape(T, H), dtype=np.float32)
    gate_w = np.asarray(gate_w, dtype=np.float32)
    w13 = np.asarray(w13, dtype=np.float32)
    w2 = np.asarray(w2, dtype=np.float32)
    sw13 = np.asarray(sw13, dtype=np.float32)
    sw2 = np.asarray(sw2, dtype=np.float32)

    combine = _gate_combine(x, gate_w)          # [T, E]

    ids = [np.nonzero(combine[:, e] > 0)[0] for e in range(E)]
    max_n = max(len(i) for i in ids)
    if max_n > C:
        # Essentially impossible for randn-style inputs (needs an 11-sigma
        # routing imbalance); fall back to an exact host computation.
        return _host_moe(x, combine, w13, w2, sw13, sw2).reshape(
            hidden_states.shape
        )
    cap = C

    if cap not in _NC_CACHE:
        _NC_CACHE[cap] = build_nc(cap)
    nc = _NC_CACHE[cap]

    xT = np.ascontiguousarray(x.T)              # [H, T]
    xts_p = _pack_xT(xT, 1024)                  # [TBP, 128, HC, 1024]

    in_maps = []
    for core in range(N_CORES):
        m = {"xts": xts_p}
        cvec = np.zeros((128, 2 * (cap // 128)), np.float32)
        for s in range(2):
            e = 2 * core + s
            tok = ids[e]
            xt_e = np.zeros((H, cap), np.float32)
            xt_e[:, : len(tok)] = xT[:, tok]
            m[f"xt{s}"] = _pack_xT(xt_e, cap)[0]
            m[f"w13_{s}"] = _pack_w13(w13[e])
            m[f"w2_{s}"] = _pack_w2(w2[e])
            cw = np.zeros(cap, np.float32)
            cw[: len(tok)] = combine[tok, e]
            cvec[:, s * (cap // 128):(s + 1) * (cap // 128)] = (
                cw.reshape(cap // 128, 128).T
            )
        m["cvec"] = cvec

        # shared expert slice (352 rows padded to ISP=384)
        lo, hi = core * 352, (core + 1) * 352
        gsl = np.zeros((ISP, H), np.float32)
        usl = np.zeros((ISP, H), np.float32)
        gsl[:352] = sw13[lo:hi]
        usl[:352] = sw13[IS + lo: IS + hi]
        # block q=2k -> gate chunk k; q=2k+1 -> up chunk k; each [128p(H), HC, 128f]
        gb = gsl.reshape(KS, 128, HC, 128).transpose(0, 3, 2, 1)
        ub = usl.reshape(KS, 128, HC, 128).transpose(0, 3, 2, 1)
        sw13_p = np.empty((2 * KS, 128, HC, 128), np.float32)
        sw13_p[0::2] = gb
        sw13_p[1::2] = ub
        m["sw13"] = np.ascontiguousarray(sw13_p)

        w2s = np.zeros((ISP, H), np.float32)
        w2s[:352] = sw2[:, lo:hi].T
        m["sw2"] = np.ascontiguousarray(
            w2s.reshape(KS, 128, HB, 512).transpose(2, 1, 0, 3)
        )
        in_maps.append(m)

    trace = bool(os.environ.get("MOE_BASS_TRACE"))
    if trace:
        _install_ntff_hook()
    res = None
    for attempt in range(3):
        try:
            res = run_bass_kernel_spmd(
                nc, in_maps, core_ids=list(range(N_CORES)), trace=trace
            )
            break
        except Exception:
            if attempt < 2:
                import time as _time

                _time.sleep(15)
    if res is None:
        # device unavailable/unrecoverable: exact (slow) host fallback
        return _host_moe(x, combine, w13, w2, sw13, sw2).reshape(
            hidden_states.shape
        )
    global LAST_EXEC_TIME_NS, LAST_TRACE
    LAST_EXEC_TIME_NS = res.exec_time_ns
    LAST_TRACE = res.instructions_and_trace

    out = np.zeros((T, H), np.float32)
    for core in range(N_CORES):
        out += res.results[core]["ys"]
        for s in range(2):
            e = 2 * core + s
            tok = ids[e]
            out[tok] += res.results[core][f"yout{s}"][: len(tok)]

    return out.reshape(hidden_states.shape).astype(np.float32)



# revision 4
# speedup vs baseline: 1.2730x; 1.2730x over previous
"""DeepseekV2 MoE (T=2048, H=2048, E=16 experts, top-6, I=1408, shared IS=2816)
on 8 Trainium2 NeuronCores.

Strategy (expert-parallel per the sharding hint):
  - Host: gate softmax/top-6 (numpy replica of the reference; the top-6/7
    logit gap is ~7e-5 so the selection is rounding-robust), per-expert token
    gather, weight transpose/packing, bf16 conversion, final scatter/combine.
  - Device (SPMD, 8 cores): experts are sorted by token count and paired
    big+small, so core i owns one "big" and one "small" expert. Slot token
    widths fd0/fd1 are the exact global maxima (rounded to 8), not a fixed
    capacity — matmul free dims stream exactly fd tokens. Plus 1/8 of the
    shared expert (tensor-parallel over the intermediate dim, 352 pad 384).
  - All matmuls in bf16 (1 cycle/row like fp32r, but FWL + background
    weight-buffer hide LDWEIGHTS, and DMA/SBUF halve). PSUM stays f32;
    rel err vs f32 reference ~4e-3 (simulated), gate is 2e-2.
  - No collectives: per-core outputs are disjoint (routed) or partial sums
    (shared) that the host adds.
"""

import os
import numpy as np
import ml_dtypes

import concourse.bass as bass
import concourse.mybir as mybir
import concourse.tile as tile
from concourse.bass_utils import run_bass_kernel_spmd

F32 = mybir.dt.float32
BF16 = mybir.dt.bfloat16
NPBF16 = ml_dtypes.bfloat16
AF = mybir.ActivationFunctionType

# problem dims (hardcoded per spec)
T, H, I, E, TOP_K = 2048, 2048, 1408, 16, 6
FF = 2 * I              # 2816
IS = 2 * I              # shared intermediate
N_CORES = 8

HC = H // 128           # 16 H chunks (contraction for mm1)
IC = I // 128           # 11 I chunks (contraction for mm2)
HB = H // 4 // 512 * 4  # noqa - keep literal below
HB = 4                  # output H blocks of 512
KS = 3                  # shared-intermediate chunks per core (352 pad 384)
ISP = 128 * KS          # 384
TBP = 2                 # shared token super-blocks (1024 tokens each)


def _blocks(fd):
    """Split a free dim into <=512 streaming blocks."""
    out, off = [], 0
    while off < fd:
        w = min(512, fd - off)
        out.append((off, w))
        off += w
    return out


def _split_excess_waits(nc, cap=1):
    """This container's walrus accepts at most one semaphore wait per
    instruction; move excess waits onto inserted same-engine NOPs."""
    for bb in nc.main_func.blocks:
        new_list = []
        for ins in bb.instructions:
            si = getattr(ins, "sync_info", None)
            waits = list(si.on_wait) if (si is not None and si.on_wait) else []
            if len(waits) > cap:
                excess, keep = waits[:-cap], waits[-cap:]
                si.on_wait = keep
                for i in range(0, len(excess), cap):
                    nop = mybir.InstNoOp(
                        name=f"I-waitsplit-{nc.next_id()}",
                        engine=ins.engine,
                        ins=[],
                        outs=[],
                        sync_info=mybir.SyncInfo(
                            on_update=[], on_wait=excess[i : i + cap]
                        ),
                        bass_nofuse=True,
                    )
                    nc.register_instruction(nop, overwrite=True)
                    new_list.append(nop)
            new_list.append(ins)
        bb.instructions = new_list


def build_nc(fd0: int, fd1: int):
    """Per-core Bass program; fd0/fd1 are slot token widths (mult of 8)."""
    fds = (fd0, fd1)
    ccn = [(fd + 127) // 128 for fd in fds]   # mm2 token chunks per slot
    nc = bass.Bass()

    # --- DRAM parameters ---
    xt_d = [
        nc.declare_dram_parameter(f"xt{s}", [128, HC, fds[s]], BF16, isOutput=False)
        for s in range(2)
    ]
    # w13[e].T blocks, order g0,u0,g1,u1,...: [2*IC][128p(H in), HC, 128]
    w13_d = [
        nc.declare_dram_parameter(f"w13_{s}", [2 * IC, 128, HC, 128], BF16, isOutput=False)
        for s in range(2)
    ]
    # w2[e].T rows chunked: [IC][128p(I in), H]
    w2_d = [
        nc.declare_dram_parameter(f"w2_{s}", [IC, 128, H], BF16, isOutput=False)
        for s in range(2)
    ]
    # x.T full (shared expert), token super-blocks: [TBP][128p(H in), HC, 1024]
    xts_d = nc.declare_dram_parameter("xts", [TBP, 128, HC, 1024], BF16, isOutput=False)
    # shared w13 slice blocks (g0,u0,g1,u1,g2,u2): [6][128p(H in), HC, 128]
    sw13_d = nc.declare_dram_parameter("sw13", [2 * KS, 128, HC, 128], BF16, isOutput=False)
    # shared w2 slice rows chunked: [KS][128p(ISP in), H]
    sw2_d = nc.declare_dram_parameter("sw2", [KS, 128, H], BF16, isOutput=False)
    # combine weights: [128, ccn0+ccn1] (col base[s]+cc -> tokens cc*128..)
    c_d = nc.declare_dram_parameter("cvec", [128, ccn[0] + ccn[1]], F32, isOutput=False)
    cbase = (0, ccn[0])

    yout_d = [
        nc.declare_dram_parameter(f"yout{s}", [fds[s], H], BF16, isOutput=True)
        for s in range(2)
    ]
    ys_d = nc.declare_dram_parameter("ys", [T, H], BF16, isOutput=True)

    with tile.TileContext(nc) as tc:
        with (
            tc.tile_pool(name="xt", bufs=1) as p_xt,
            tc.tile_pool(name="w13", bufs=3) as p_w13,
            tc.tile_pool(name="wres", bufs=1) as p_wres,
            tc.tile_pool(name="tmp", bufs=3) as p_tmp,
            tc.tile_pool(name="aT", bufs=1) as p_aT,
            tc.tile_pool(name="y", bufs=4) as p_y,
            tc.tile_pool(name="c", bufs=1) as p_c,
            tc.tile_pool(name="ps", bufs=8, space="PSUM") as p_ps,
        ):
            c_sb = p_c.tile([128, ccn[0] + ccn[1]], F32)
            nc.sync.dma_start(out=c_sb[:], in_=c_d[:])

            # shared-expert weights: resident for both phases
            sw13_sb = p_wres.tile([128, 2 * KS, HC, 128], BF16, tag="sw13")
            for q in range(2 * KS):
                nc.sync.dma_start(out=sw13_sb[:, q], in_=sw13_d[q])
            sw2_sb = p_wres.tile([128, KS, H], BF16, tag="sw2")
            for k in range(KS):
                nc.sync.dma_start(out=sw2_sb[:, k], in_=sw2_d[k])

            def load_xt(dram_src, width, tag):
                """Per-H-chunk strip DMAs so the first matmuls start early."""
                t = p_xt.tile([128, HC, width], BF16, tag=tag)
                for hc in range(HC):
                    nc.sync.dma_start(out=t[:, hc, :], in_=dram_src[:, hc, :])
                return t

            def swiglu_mm1(xt_sb, get_w, n_i, aT_sb, fd):
                """mm1 + SiLU*u. xt_sb: [128, HC, fd]; get_w(i) -> (wg, wu)
                stationary tiles [128, HC, 128]; aT_sb dest [128, n_i, fd]."""
                for i in range(n_i):
                    wg, wu = get_w(i)
                    for off, w in _blocks(fd):
                        col = slice(off, off + w)
                        ps_g = p_ps.tile([128, 512], F32, tag="ps")
                        for hc in range(HC):
                            nc.tensor.matmul(
                                ps_g[:, :w], wg[:, hc, :], xt_sb[:, hc, col],
                                start=(hc == 0), stop=(hc == HC - 1),
                            )
                        ps_u = p_ps.tile([128, 512], F32, tag="ps")
                        for hc in range(HC):
                            nc.tensor.matmul(
                                ps_u[:, :w], wu[:, hc, :], xt_sb[:, hc, col],
                                start=(hc == 0), stop=(hc == HC - 1),
                            )
                        tmp = p_tmp.tile([128, 512], F32, tag="tmp")
                        nc.scalar.activation(
                            out=tmp[:, :w], in_=ps_g[:, :w], func=AF.Silu
                        )
                        nc.vector.tensor_mul(
                            out=aT_sb[:, i, col], in0=tmp[:, :w], in1=ps_u[:, :w]
                        )

            def expert_phase(s):
                fd = fds[s]
                xt_sb = load_xt(xt_d[s], fd, tag="xt")

                def get_w(i):
                    wg = p_w13.tile([128, HC, 128], BF16, tag="w13")
                    nc.sync.dma_start(out=wg[:], in_=w13_d[s][2 * i])
                    wu = p_w13.tile([128, HC, 128], BF16, tag="w13")
                    nc.sync.dma_start(out=wu[:], in_=w13_d[s][2 * i + 1])
                    return wg, wu

                aT = p_aT.tile([128, IC, fd], BF16, tag=f"aT{s}")
                swiglu_mm1(xt_sb, get_w, IC, aT, fd)

                # mm2: w2 rows resident; stationary aT token-chunk serves all
                # four H blocks (amortizes each weight load over 4 matmuls)
                w2_sb = p_wres.tile([128, IC, H], BF16, tag="w2res")
                for ic in range(IC):
                    nc.sync.dma_start(out=w2_sb[:, ic], in_=w2_d[s][ic])
                for cc in range(ccn[s]):
                    t0 = cc * 128
                    rows = min(128, fd - t0)
                    ps_y = []
                    for hb in range(HB):
                        ps_t = p_ps.tile([128, 512], F32, tag="ps")
                        ps_y.append(ps_t)
                    for ic in range(IC):
                        st = aT[:, ic, t0:t0 + rows]
                        for hb in range(HB):
                            nc.tensor.matmul(
                                ps_y[hb][:rows, :], st,
                                w2_sb[:, ic, hb * 512:(hb + 1) * 512],
                                start=(ic == 0), stop=(ic == IC - 1),
                            )
                    for hb in range(HB):
                        y_sb = p_y.tile([128, 512], BF16, tag="y")
                        nc.vector.tensor_scalar_mul(
                            y_sb[:rows, :], ps_y[hb][:rows, :],
                            c_sb[:rows, cbase[s] + cc: cbase[s] + cc + 1],
                        )
                        nc.gpsimd.dma_start(
                            out=yout_d[s][t0:t0 + rows, hb * 512:(hb + 1) * 512],
                            in_=y_sb[:rows, :],
                        )

            def shared_phase(tbp):
                xts_sb = load_xt(xts_d[tbp], 1024, tag="xts")

                def get_w(i):
                    return sw13_sb[:, 2 * i], sw13_sb[:, 2 * i + 1]

                aTs = p_aT.tile([128, KS, 1024], BF16, tag="aTs")
                swiglu_mm1(xts_sb, get_w, KS, aTs, 1024)

                for tc_ in range(8):
                    st_col = slice(tc_ * 128, (tc_ + 1) * 128)
                    ps_y = []
                    for hb in range(HB):
                        ps_t = p_ps.tile([128, 512], F32, tag="ps")
                        ps_y.append(ps_t)
                    for k in range(KS):
                        st = aTs[:, k, st_col]
                        for hb in range(HB):
                            nc.tensor.matmul(
                                ps_y[hb][:], st,
                                sw2_sb[:, k, hb * 512:(hb + 1) * 512],
                                start=(k == 0), stop=(k == KS - 1),
                            )
                    row0 = tbp * 1024 + tc_ * 128
                    for hb in range(HB):
                        y_sb = p_y.tile([128, 512], BF16, tag="y")
                        nc.scalar.copy(y_sb[:], ps_y[hb][:])
                        nc.gpsimd.dma_start(
                            out=ys_d[row0:row0 + 128, hb * 512:(hb + 1) * 512],
                            in_=y_sb[:],
                        )

            expert_phase(0)
            shared_phase(0)
            expert_phase(1)
            shared_phase(1)

    _split_excess_waits(nc, cap=1)
    return nc


# ------------------------- host side -------------------------

def _gate_combine(x, gate_w):
    """Replica of the reference gate in pure numpy (f32). The top-6 selection
    is what must match the reference exactly; the smallest rank-6/rank-7 logit
    gap over the 2048 tokens is ~7e-5 while cross-implementation f32 rounding
    differences are ~1e-6, so the selection is identical. Tie-break on exact
    equality follows lax.top_k (lowest index wins)."""
    z = (x @ gate_w.T).astype(np.float32)                 # [T, E] logits
    z64 = z.astype(np.float64)
    m = z64.max(-1, keepdims=True)
    ez = np.exp(z64 - m)
    scores = (ez / ez.sum(-1, keepdims=True)).astype(np.float32)
    order = np.argsort(-scores, axis=-1, kind="stable")[:, :TOP_K]
    topk_w = np.take_along_axis(scores, order, axis=-1)
    topk_w = topk_w / (topk_w.sum(-1, keepdims=True) + 1e-20)
    combine = np.zeros((x.shape[0], E), np.float32)
    np.put_along_axis(combine, order, topk_w, axis=-1)
    return combine


def _pack_w13(w13e):
    """w13[e] [FF, H] -> [2*IC, 128, HC, 128] bf16, block order g0,u0,..."""
    a = np.ascontiguousarray(
        w13e.astype(NPBF16).reshape(2 * IC, 128, HC, 128).transpose(0, 3, 2, 1)
    )
    order = np.empty(2 * IC, np.int64)
    order[0::2] = np.arange(IC)
    order[1::2] = np.arange(IC) + IC
    return np.ascontiguousarray(a[order])


def _pack_w2(w2e):
    """w2[e] [H, I] -> [IC, 128, H] bf16: w2T rows chunked by 128."""
    return np.ascontiguousarray(w2e.T.astype(NPBF16).reshape(IC, 128, H))


def _pack_xT(xT, width):
    """xT [H, n*width] f32 -> [n, 128, HC, width] bf16"""
    n = xT.shape[1] // width
    return np.ascontiguousarray(
        xT.astype(NPBF16).reshape(HC, 128, n, width).transpose(2, 1, 0, 3)
    )


def _host_moe(x, combine, w13, w2, sw13, sw2):
    """Exact numpy fallback (only used if the device run fails)."""

    def silu(v):
        return v / (1.0 + np.exp(-v))

    out = np.zeros((T, H), np.float32)
    for e in range(E):
        gu = x @ w13[e].T
        a = silu(gu[:, :I]) * gu[:, I:]
        out += combine[:, e:e + 1] * (a @ w2[e].T)
    gu = x @ sw13.T
    a = silu(gu[:, :IS]) * gu[:, IS:]
    out += a @ sw2.T
    return out


_NC_CACHE = {}

LAST_EXEC_TIME_NS = None
LAST_TRACE = None


def _install_ntff_hook():
    """Bridge the missing ``antenv.axon_hooks`` module so trace=True works
    in this container (used by test.py only; harmless if already present)."""
    import sys, types

    try:
        from antenv.axon_hooks import get_axon_ntff_profile_hook  # noqa: F401
        return
    except ImportError:
        pass
    import antenv  # noqa: F401
    import trn_agent_boot.trn_boot as tb

    mod = types.ModuleType("antenv.axon_hooks")
    _h = [None]
    mod.set_axon_ntff_profile_hook = lambda h: _h.__setitem__(0, h)
    mod.get_axon_ntff_profile_hook = lambda: _h[0]
    sys.modules["antenv.axon_hooks"] = mod
    mod.set_axon_ntff_profile_hook(
        tb._ntff_profile_via_ctypes("/opt/axon/libaxon_pjrt.so")
    )


def kernel(hidden_states, gate_w, w13, w2, sw13, sw2):
    hidden_states = np.asarray(hidden_states)
    x = np.ascontiguousarray(hidden_states.reshape(T, H), dtype=np.float32)
    gate_w = np.asarray(gate_w, dtype=np.float32)
    w13 = np.asarray(w13, dtype=np.float32)
    w2 = np.asarray(w2, dtype=np.float32)
    sw13 = np.asarray(sw13, dtype=np.float32)
    sw2 = np.asarray(sw2, dtype=np.float32)

    combine = _gate_combine(x, gate_w)          # [T, E]

    ids = [np.nonzero(combine[:, e] > 0)[0] for e in range(E)]
    counts = np.array([len(i) for i in ids])
    order = np.argsort(-counts, kind="stable")
    slot_exp = [list(order[:8]), list(order[8:][::-1])]   # big slot, small slot
    fd0 = max(128, -(-int(counts[order[0]]) // 8) * 8)
    fd1 = max(128, -(-int(counts[order[8]]) // 8) * 8)
    ccn = [(fd0 + 127) // 128, (fd1 + 127) // 128]
    fds = (fd0, fd1)

    key = (fd0, fd1)
    if key not in _NC_CACHE:
        _NC_CACHE[key] = build_nc(fd0, fd1)
    nc = _NC_CACHE[key]

    xT = np.ascontiguousarray(x.T)              # [H, T] f32
    xts_p = _pack_xT(xT, 1024)                  # [TBP, 128, HC, 1024] bf16

    # shared-expert per-core slices (built once, indexed per core)
    in_maps = []
    for core in range(N_CORES):
        m = {"xts": xts_p}
        cvec = np.zeros((128, ccn[0] + ccn[1]), np.float32)
        for s in range(2):
            e = int(slot_exp[s][core])
            fd = fds[s]
            tok = ids[e]
            xt_e = np.zeros((H, fd), np.float32)
            xt_e[:, : len(tok)] = xT[:, tok]
            m[f"xt{s}"] = _pack_xT(xt_e, fd)[0]
            m[f"w13_{s}"] = _pack_w13(w13[e])
            m[f"w2_{s}"] = _pack_w2(w2[e])
            cw = np.zeros(ccn[s] * 128, np.float32)
            cw[: len(tok)] = combine[tok, e]
            base = 0 if s == 0 else ccn[0]
            cvec[:, base:base + ccn[s]] = cw.reshape(ccn[s], 128).T
        m["cvec"] = cvec

        # shared expert slice (352 rows padded to ISP=384)
        lo, hi = core * 352, (core + 1) * 352
        gsl = np.zeros((ISP, H), np.float32)
        usl = np.zeros((ISP, H), np.float32)
        gsl[:352] = sw13[lo:hi]
        usl[:352] = sw13[IS + lo: IS + hi]
        gb = gsl.astype(NPBF16).reshape(KS, 128, HC, 128).transpose(0, 3, 2, 1)
        ub = usl.astype(NPBF16).reshape(KS, 128, HC, 128).transpose(0, 3, 2, 1)
        sw13_p = np.empty((2 * KS, 128, HC, 128), NPBF16)
        sw13_p[0::2] = gb
        sw13_p[1::2] = ub
        m["sw13"] = np.ascontiguousarray(sw13_p)

        w2s = np.zeros((ISP, H), np.float32)
        w2s[:352] = sw2[:, lo:hi].T
        m["sw2"] = np.ascontiguousarray(w2s.astype(NPBF16).reshape(KS, 128, H))
        in_maps.append(m)

    trace = bool(os.environ.get("MOE_BASS_TRACE"))
    if trace:
        _install_ntff_hook()
    res = None
    for attempt in range(3):
        try:
            res = run_bass_kernel_spmd(
                nc, in_maps, core_ids=list(range(N_CORES)), trace=trace
            )
            break
        except Exception:
            if attempt < 2:
                import time as _time

                _time.sleep(15)
    if res is None:
        # device unavailable/unrecoverable: exact (slow) host fallback
        return _host_moe(x, combine, w13, w2, sw13, sw2).reshape(
            hidden_states.shape
        )
    global LAST_EXEC_TIME_NS, LAST_TRACE
    LAST_EXEC_TIME_NS = res.exec_time_ns
    LAST_TRACE = res.instructions_and_trace

    out = np.zeros((T, H), np.float32)
    for core in range(N_CORES):
        out += res.results[core]["ys"].astype(np.float32)
        for s in range(2):
            e = int(slot_exp[s][core])
            tok = ids[e]
            out[tok] += res.results[core][f"yout{s}"][: len(tok)].astype(
                np.float32
            )

    return out.reshape(hidden_states.shape).astype(np.float32)


# revision 6
# speedup vs baseline: 1.3033x; 1.0239x over previous
"""DeepseekV2 MoE (T=2048, H=2048, E=16 experts, top-6, I=1408, shared IS=2816)
on 8 Trainium2 NeuronCores.

Strategy (expert-parallel per the sharding hint):
  - Host: gate softmax/top-6 (numpy replica of the reference; the top-6/7
    logit gap is ~7e-5 so the selection is rounding-robust), per-expert token
    gather, weight transpose/packing, bf16 conversion, final scatter/combine.
  - Device (SPMD, 8 cores): experts are sorted by token count and paired
    big+small, so core i owns one "big" and one "small" expert. Slot token
    widths fd0/fd1 are the exact global maxima (rounded to 8), not a fixed
    capacity — matmul free dims stream exactly fd tokens. Plus 1/8 of the
    shared expert (tensor-parallel over the intermediate dim, 352 pad 384).
  - All matmuls in bf16 (1 cycle/row like fp32r, but FWL + background
    weight-buffer hide LDWEIGHTS, and DMA/SBUF halve). PSUM stays f32;
    rel err vs f32 reference ~4e-3 (simulated), gate is 2e-2.
  - No collectives: per-core outputs are disjoint (routed) or partial sums
    (shared) that the host adds.
"""

import os
import numpy as np
import ml_dtypes

import concourse.bass as bass
import concourse.mybir as mybir
import concourse.tile as tile
from concourse.bass_utils import run_bass_kernel_spmd

F32 = mybir.dt.float32
BF16 = mybir.dt.bfloat16
NPBF16 = ml_dtypes.bfloat16
AF = mybir.ActivationFunctionType

# problem dims (hardcoded per spec)
T, H, I, E, TOP_K = 2048, 2048, 1408, 16, 6
FF = 2 * I              # 2816
IS = 2 * I              # shared intermediate
N_CORES = 8

HC = H // 128           # 16 H chunks (contraction for mm1)
IC = I // 128           # 11 I chunks (contraction for mm2)
HB = H // 4 // 512 * 4  # noqa - keep literal below
HB = 4                  # output H blocks of 512
KS = 3                  # shared-intermediate chunks per core (352 pad 384)
ISP = 128 * KS          # 384
TBP = 2                 # shared token super-blocks (1024 tokens each)


def _blocks(fd):
    """Split a free dim into <=512 streaming blocks."""
    out, off = [], 0
    while off < fd:
        w = min(512, fd - off)
        out.append((off, w))
        off += w
    return out


def _split_excess_waits(nc, cap=1):
    """This container's walrus accepts at most one semaphore wait per
    instruction; move excess waits onto inserted same-engine NOPs."""
    for bb in nc.main_func.blocks:
        new_list = []
        for ins in bb.instructions:
            si = getattr(ins, "sync_info", None)
            waits = list(si.on_wait) if (si is not None and si.on_wait) else []
            if len(waits) > cap:
                excess, keep = waits[:-cap], waits[-cap:]
                si.on_wait = keep
                for i in range(0, len(excess), cap):
                    nop = mybir.InstNoOp(
                        name=f"I-waitsplit-{nc.next_id()}",
                        engine=ins.engine,
                        ins=[],
                        outs=[],
                        sync_info=mybir.SyncInfo(
                            on_update=[], on_wait=excess[i : i + cap]
                        ),
                        bass_nofuse=True,
                    )
                    nc.register_instruction(nop, overwrite=True)
                    new_list.append(nop)
            new_list.append(ins)
        bb.instructions = new_list


def build_nc(fd0: int, fd1: int):
    """Per-core Bass program; fd0/fd1 are slot token widths (mult of 8)."""
    fds = (fd0, fd1)
    ccn = [(fd + 127) // 128 for fd in fds]   # mm2 token chunks per slot
    nc = bass.Bass()

    # --- DRAM parameters ---
    xt_d = [
        nc.declare_dram_parameter(f"xt{s}", [128, HC, fds[s]], BF16, isOutput=False)
        for s in range(2)
    ]
    # w13[e].T blocks, order g0,u0,g1,u1,...: [2*IC][128p(H in), HC, 128]
    w13_d = [
        nc.declare_dram_parameter(f"w13_{s}", [2 * IC, 128, HC, 128], BF16, isOutput=False)
        for s in range(2)
    ]
    # w2[e].T rows chunked: [IC][128p(I in), H]
    w2_d = [
        nc.declare_dram_parameter(f"w2_{s}", [IC, 128, H], BF16, isOutput=False)
        for s in range(2)
    ]
    # x.T full (shared expert), token super-blocks: [TBP][128p(H in), HC, 1024]
    xts_d = nc.declare_dram_parameter("xts", [TBP, 128, HC, 1024], BF16, isOutput=False)
    # shared w13 slice blocks (g0,u0,g1,u1,g2,u2): [6][128p(H in), HC, 128]
    sw13_d = nc.declare_dram_parameter("sw13", [2 * KS, 128, HC, 128], BF16, isOutput=False)
    # shared w2 slice rows chunked: [KS][128p(ISP in), H]
    sw2_d = nc.declare_dram_parameter("sw2", [KS, 128, H], BF16, isOutput=False)
    # combine weights: [128, ccn0+ccn1] (col base[s]+cc -> tokens cc*128..)
    c_d = nc.declare_dram_parameter("cvec", [128, ccn[0] + ccn[1]], F32, isOutput=False)
    cbase = (0, ccn[0])

    yout_d = [
        nc.declare_dram_parameter(f"yout{s}", [fds[s], H], BF16, isOutput=True)
        for s in range(2)
    ]
    ys_d = nc.declare_dram_parameter("ys", [T, H], BF16, isOutput=True)

    with tile.TileContext(nc) as tc:
        with (
            tc.tile_pool(name="xt", bufs=1) as p_xt,
            tc.tile_pool(name="w13", bufs=3) as p_w13,
            tc.tile_pool(name="wres", bufs=1) as p_wres,
            tc.tile_pool(name="tmp", bufs=3) as p_tmp,
            tc.tile_pool(name="aT", bufs=1) as p_aT,
            tc.tile_pool(name="y", bufs=4) as p_y,
            tc.tile_pool(name="c", bufs=1) as p_c,
            tc.tile_pool(name="ps", bufs=8, space="PSUM") as p_ps,
        ):
            c_sb = p_c.tile([128, ccn[0] + ccn[1]], F32)
            nc.sync.dma_start(out=c_sb[:], in_=c_d[:])

            # shared-expert weights: resident for both phases; loads are
            # issued after expert 0's (see below) so the first matmul's
            # inputs hit the head of the DMA queue.
            sw13_sb = p_wres.tile([128, 2 * KS, HC, 128], BF16, tag="sw13")
            sw2_sb = p_wres.tile([128, KS, H], BF16, tag="sw2")

            def load_shared_weights():
                for q in range(2 * KS):
                    nc.sync.dma_start(out=sw13_sb[:, q], in_=sw13_d[q])
                for k in range(KS):
                    nc.sync.dma_start(out=sw2_sb[:, k], in_=sw2_d[k])

            def load_xt(dram_src, width, tag):
                """Per-H-chunk strip DMAs so the first matmuls start early."""
                t = p_xt.tile([128, HC, width], BF16, tag=tag)
                for hc in range(HC):
                    nc.sync.dma_start(out=t[:, hc, :], in_=dram_src[:, hc, :])
                return t

            def swiglu_mm1(xt_sb, get_w, n_i, aT_sb, fd):
                """mm1 + SiLU*u. xt_sb: [128, HC, fd]; get_w(i) -> (wg, wu)
                stationary tiles [128, HC, 128]; aT_sb dest [128, n_i, fd]."""
                for i in range(n_i):
                    wg, wu = get_w(i)
                    for off, w in _blocks(fd):
                        col = slice(off, off + w)
                        ps_g = p_ps.tile([128, 512], F32, tag="ps")
                        for hc in range(HC):
                            nc.tensor.matmul(
                                ps_g[:, :w], wg[:, hc, :], xt_sb[:, hc, col],
                                start=(hc == 0), stop=(hc == HC - 1),
                            )
                        ps_u = p_ps.tile([128, 512], F32, tag="ps")
                        for hc in range(HC):
                            nc.tensor.matmul(
                                ps_u[:, :w], wu[:, hc, :], xt_sb[:, hc, col],
                                start=(hc == 0), stop=(hc == HC - 1),
                            )
                        tmp = p_tmp.tile([128, 512], F32, tag="tmp")
                        nc.scalar.activation(
                            out=tmp[:, :w], in_=ps_g[:, :w], func=AF.Silu
                        )
                        nc.vector.tensor_mul(
                            out=aT_sb[:, i, col], in0=tmp[:, :w], in1=ps_u[:, :w]
                        )

            def expert_phase(s):
                fd = fds[s]
                xt_sb = load_xt(xt_d[s], fd, tag="xt")

                def get_w(i):
                    wg = p_w13.tile([128, HC, 128], BF16, tag="w13")
                    nc.sync.dma_start(out=wg[:], in_=w13_d[s][2 * i])
                    wu = p_w13.tile([128, HC, 128], BF16, tag="w13")
                    nc.sync.dma_start(out=wu[:], in_=w13_d[s][2 * i + 1])
                    return wg, wu

                aT = p_aT.tile([128, IC, fd], BF16, tag=f"aT{s}")
                swiglu_mm1(xt_sb, get_w, IC, aT, fd)

                # mm2: w2 rows resident; stationary aT token-chunk serves all
                # four H blocks (amortizes each weight load over 4 matmuls)
                w2_sb = p_wres.tile([128, IC, H], BF16, tag="w2res")
                for ic in range(IC):
                    nc.sync.dma_start(out=w2_sb[:, ic], in_=w2_d[s][ic])
                for cc in range(ccn[s]):
                    t0 = cc * 128
                    rows = min(128, fd - t0)
                    ps_y = []
                    for hb in range(HB):
                        ps_t = p_ps.tile([128, 512], F32, tag="ps")
                        ps_y.append(ps_t)
                    for ic in range(IC):
                        st = aT[:, ic, t0:t0 + rows]
                        for hb in range(HB):
                            nc.tensor.matmul(
                                ps_y[hb][:rows, :], st,
                                w2_sb[:, ic, hb * 512:(hb + 1) * 512],
                                start=(ic == 0), stop=(ic == IC - 1),
                            )
                    for hb in range(HB):
                        y_sb = p_y.tile([128, 512], BF16, tag="y")
                        nc.vector.tensor_scalar_mul(
                            y_sb[:rows, :], ps_y[hb][:rows, :],
                            c_sb[:rows, cbase[s] + cc: cbase[s] + cc + 1],
                        )
                        nc.gpsimd.dma_start(
                            out=yout_d[s][t0:t0 + rows, hb * 512:(hb + 1) * 512],
                            in_=y_sb[:rows, :],
                        )

            def shared_phase(tbp):
                xts_sb = load_xt(xts_d[tbp], 1024, tag="xts")

                def get_w(i):
                    return sw13_sb[:, 2 * i], sw13_sb[:, 2 * i + 1]

                aTs = p_aT.tile([128, KS, 1024], BF16, tag="aTs")
                swiglu_mm1(xts_sb, get_w, KS, aTs, 1024)

                for tc_ in range(8):
                    st_col = slice(tc_ * 128, (tc_ + 1) * 128)
                    ps_y = []
                    for hb in range(HB):
                        ps_t = p_ps.tile([128, 512], F32, tag="ps")
                        ps_y.append(ps_t)
                    for k in range(KS):
                        st = aTs[:, k, st_col]
                        for hb in range(HB):
                            nc.tensor.matmul(
                                ps_y[hb][:], st,
                                sw2_sb[:, k, hb * 512:(hb + 1) * 512],
                                start=(k == 0), stop=(k == KS - 1),
                            )
                    row0 = tbp * 1024 + tc_ * 128
                    for hb in range(HB):
                        y_sb = p_y.tile([128, 512], BF16, tag="y")
                        nc.scalar.copy(y_sb[:], ps_y[hb][:])
                        nc.gpsimd.dma_start(
                            out=ys_d[row0:row0 + 128, hb * 512:(hb + 1) * 512],
                            in_=y_sb[:],
                        )

            expert_phase(0)
            load_shared_weights()
            shared_phase(0)
            expert_phase(1)
            shared_phase(1)

    _split_excess_waits(nc, cap=1)
    return nc


# ------------------------- host side -------------------------

def _gate_combine(x, gate_w):
    """Replica of the reference gate in pure numpy (f32). The top-6 selection
    is what must match the reference exactly; the smallest rank-6/rank-7 logit
    gap over the 2048 tokens is ~7e-5 while cross-implementation f32 rounding
    differences are ~1e-6, so the selection is identical. Tie-break on exact
    equality follows lax.top_k (lowest index wins)."""
    z = (x @ gate_w.T).astype(np.float32)                 # [T, E] logits
    z64 = z.astype(np.float64)
    m = z64.max(-1, keepdims=True)
    ez = np.exp(z64 - m)
    scores = (ez / ez.sum(-1, keepdims=True)).astype(np.float32)
    order = np.argsort(-scores, axis=-1, kind="stable")[:, :TOP_K]
    topk_w = np.take_along_axis(scores, order, axis=-1)
    topk_w = topk_w / (topk_w.sum(-1, keepdims=True) + 1e-20)
    combine = np.zeros((x.shape[0], E), np.float32)
    np.put_along_axis(combine, order, topk_w, axis=-1)
    return combine


def _pack_w13(w13e):
    """w13[e] [FF, H] -> [2*IC, 128, HC, 128] bf16, block order g0,u0,..."""
    a = np.ascontiguousarray(
        w13e.astype(NPBF16).reshape(2 * IC, 128, HC, 128).transpose(0, 3, 2, 1)
    )
    order = np.empty(2 * IC, np.int64)
    order[0::2] = np.arange(IC)
    order[1::2] = np.arange(IC) + IC
    return np.ascontiguousarray(a[order])


def _pack_w2(w2e):
    """w2[e] [H, I] -> [IC, 128, H] bf16: w2T rows chunked by 128."""
    return np.ascontiguousarray(w2e.T.astype(NPBF16).reshape(IC, 128, H))


def _pack_xT(xT, width):
    """xT [H, n*width] f32 -> [n, 128, HC, width] bf16"""
    n = xT.shape[1] // width
    return np.ascontiguousarray(
        xT.astype(NPBF16).reshape(HC, 128, n, width).transpose(2, 1, 0, 3)
    )


def _host_moe(x, combine, w13, w2, sw13, sw2):
    """Exact numpy fallback (only used if the device run fails)."""

    def silu(v):
        return v / (1.0 + np.exp(-v))

    out = np.zeros((T, H), np.float32)
    for e in range(E):
        gu = x @ w13[e].T
        a = silu(gu[:, :I]) * gu[:, I:]
        out += combine[:, e:e + 1] * (a @ w2[e].T)
    gu = x @ sw13.T
    a = silu(gu[:, :IS]) * gu[:, IS:]
    out += a @ sw2.T
    return out


_NC_CACHE = {}

LAST_EXEC_TIME_NS = None
LAST_TRACE = None


def _install_ntff_hook():
    """Bridge the missing ``antenv.axon_hooks`` module so trace=True works
    in this container (used by test.py only; harmless if already present)."""
    import sys, types

    try:
        from antenv.axon_hooks import get_axon_ntff_profile_hook  # noqa: F401
        return
    except ImportError:
        pass
    import antenv  # noqa: F401
    import trn_agent_boot.trn_boot as tb

    mod = types.ModuleType("antenv.axon_hooks")
    _h = [None]
    mod.set_axon_ntff_profile_hook = lambda h: _h.__setitem__(0, h)
    mod.get_axon_ntff_profile_hook = lambda: _h[0]
    sys.modules["antenv.axon_hooks"] = mod
    mod.set_axon_ntff_profile_hook(
        tb._ntff_profile_via_ctypes("/opt/axon/libaxon_pjrt.so")
    )


def kernel(hidden_states, gate_w, w13, w2, sw13, sw2):
    hidden_states = np.asarray(hidden_states)
    x = np.ascontiguousarray(hidden_states.reshape(T, H), dtype=np.float32)
    gate_w = np.asarray(gate_w, dtype=np.float32)
    w13 = np.asarray(w13, dtype=np.float32)
    w2 = np.asarray(w2, dtype=np.float32)
    sw13 = np.asarray(sw13, dtype=np.float32)
    sw2 = np.asarray(sw2, dtype=np.float32)

    combine = _gate_combine(x, gate_w)          # [T, E]

    ids = [np.nonzero(combine[:, e] > 0)[0] for e in range(E)]
    counts = np.array([len(i) for i in ids])
    order = np.argsort(-counts, kind="stable")
    slot_exp = [list(order[:8]), list(order[8:][::-1])]   # big slot, small slot
    fd0 = max(128, -(-int(counts[order[0]]) // 8) * 8)
    fd1 = max(128, -(-int(counts[order[8]]) // 8) * 8)
    ccn = [(fd0 + 127) // 128, (fd1 + 127) // 128]
    fds = (fd0, fd1)

    key = (fd0, fd1)
    if key not in _NC_CACHE:
        _NC_CACHE[key] = build_nc(fd0, fd1)
    nc = _NC_CACHE[key]

    xT = np.ascontiguousarray(x.T)              # [H, T] f32
    xts_p = _pack_xT(xT, 1024)                  # [TBP, 128, HC, 1024] bf16

    # shared-expert per-core slices (built once, indexed per core)
    in_maps = []
    for core in range(N_CORES):
        m = {"xts": xts_p}
        cvec = np.zeros((128, ccn[0] + ccn[1]), np.float32)
        for s in range(2):
            e = int(slot_exp[s][core])
            fd = fds[s]
            tok = ids[e]
            xt_e = np.zeros((H, fd), np.float32)
            xt_e[:, : len(tok)] = xT[:, tok]
            m[f"xt{s}"] = _pack_xT(xt_e, fd)[0]
            m[f"w13_{s}"] = _pack_w13(w13[e])
            m[f"w2_{s}"] = _pack_w2(w2[e])
            cw = np.zeros(ccn[s] * 128, np.float32)
            cw[: len(tok)] = combine[tok, e]
            base = 0 if s == 0 else ccn[0]
            cvec[:, base:base + ccn[s]] = cw.reshape(ccn[s], 128).T
        m["cvec"] = cvec

        # shared expert slice (352 rows padded to ISP=384)
        lo, hi = core * 352, (core + 1) * 352
        gsl = np.zeros((ISP, H), np.float32)
        usl = np.zeros((ISP, H), np.float32)
        gsl[:352] = sw13[lo:hi]
        usl[:352] = sw13[IS + lo: IS + hi]
        gb = gsl.astype(NPBF16).reshape(KS, 128, HC, 128).transpose(0, 3, 2, 1)
        ub = usl.astype(NPBF16).reshape(KS, 128, HC, 128).transpose(0, 3, 2, 1)
        sw13_p = np.empty((2 * KS, 128, HC, 128), NPBF16)
        sw13_p[0::2] = gb
        sw13_p[1::2] = ub
        m["sw13"] = np.ascontiguousarray(sw13_p)

        w2s = np.zeros((ISP, H), np.float32)
        w2s[:352] = sw2[:, lo:hi].T
        m["sw2"] = np.ascontiguousarray(w2s.astype(NPBF16).reshape(KS, 128, H))
        in_maps.append(m)

    trace = bool(os.environ.get("MOE_BASS_TRACE"))
    if trace:
        _install_ntff_hook()
    res = None
    for attempt in range(3):
        try:
            res = run_bass_kernel_spmd(
                nc, in_maps, core_ids=list(range(N_CORES)), trace=trace
            )
            break
        except Exception:
            if attempt < 2:
                import time as _time

                _time.sleep(15)
    if res is None:
        # device unavailable/unrecoverable: exact (slow) host fallback
        return _host_moe(x, combine, w13, w2, sw13, sw2).reshape(
            hidden_states.shape
        )
    global LAST_EXEC_TIME_NS, LAST_TRACE
    LAST_EXEC_TIME_NS = res.exec_time_ns
    LAST_TRACE = res.instructions_and_trace

    out = np.zeros((T, H), np.float32)
    for core in range(N_CORES):
        out += res.results[core]["ys"].astype(np.float32)
        for s in range(2):
            e = int(slot_exp[s][core])
            tok = ids[e]
            out[tok] += res.results[core][f"yout{s}"][: len(tok)].astype(
                np.float32
            )

    return out.reshape(hidden_states.shape).astype(np.float32)
